# revision 37
# baseline (speedup 1.0000x reference)
"""Self-attention kernel for TRN2: out = softmax(X Wq (X Wk)^T / sqrt(D)) @ X.

Strategy (8-way sequence parallelism over query rows):
  scores = (X Wq)(X Wk)^T = X M X^T  with  M = (Wq/sqrt(D)) Wk^T
so K is never materialized. Each core i handles query rows [i*B, (i+1)*B):
  phase 0: M = Wqs Wk^T (fp32), A_i^T = M^T X_i^T (fp32), both on-device.
  flash:   stream key blocks j; S^T_j logits in key-major layout via three
           f32r matmul streams (hi/lo splits of X precomputed on the host,
           A split on-device); running column-max via PE transpose + reduce;
           E = exp(S - max) written as f32r by the ACT engine; out
           accumulation E^T-slices @ X (f32r, host-prequantized, direct
           DMA); softmax denominator via ones-vector-stationary matmuls
           into a [1,B] PSUM row, folded back to query-major via DMA + PE
           transpose; fused rescale-accumulate (acc = acc*corr + psum).

Numerics: logits need ~fp32 precision (std ~1024, near-tie rows amplify
errors through softmax), so the S matmul uses 3 f32r streams
(Xh Ah + Xh Al + Xl Ah). f32r is fp32 with the low 12 mantissa bits
rounded away, so host-prerounded splits survive DMA bit-exactly. The P@X
matmul only needs ~1e-3 relative, so single f32r streams are safe there.
"""
import numpy as np
from contextlib import ExitStack

import concourse.bass as bass
import concourse.bacc as bacc
import concourse.tile as tile
from concourse import mybir
from concourse.bass_utils import run_bass_kernel_spmd
from concourse.masks import make_identity

P = 128
SEQ = 8192
DIM = 1024
NCORES = 8
SBN = 4      # key n-tiles (of 128) per flash super-block

F32 = mybir.dt.float32
F32R = mybir.dt.float32r
EXP = mybir.ActivationFunctionType.Exp
ALU = mybir.AluOpType
AXX = mybir.AxisListType.X


def _chunks(total, step=512):
    return [(lo, min(lo + step, total)) for lo in range(0, total, step)]


def rne12(x):
    """Round fp32 to f32r precision (drop 12 mantissa bits, RNE)."""
    u = np.ascontiguousarray(x, np.float32).view(np.uint32)
    half = np.uint32(0x800)
    lsb = (u >> np.uint32(12)) & np.uint32(1)
    u2 = (u + half - np.uint32(1) + lsb) & np.uint32(0xFFFFF000)
    return u2.view(np.float32)


def build_core_kernel(S, D, B, sbn=SBN, debug=False):
    """One core's kernel: query rows block of size B, full S keys."""
    KT = D // P      # contraction tiles over D
    NT = S // P      # key tiles
    MT = B // P      # query tiles (per core)
    NSB = NT // sbn  # super-blocks
    assert NT % sbn == 0 and B % P == 0 and D % P == 0 and MT <= P

    nc = bacc.Bacc("TRN2", target_bir_lowering=False, debug=False)
    xtjh = nc.dram_tensor("xtjh", [NT, P, D], F32R, kind="ExternalInput")
    xtjl = nc.dram_tensor("xtjl", [NT, P, D], F32R, kind="ExternalInput")
    xa = nc.dram_tensor("xa", [S, D], F32R, kind="ExternalInput")
    wqst = nc.dram_tensor("wqst", [D, D], F32, kind="ExternalInput")
    wkt = nc.dram_tensor("wkt", [D, D], F32, kind="ExternalInput")
    xit = nc.dram_tensor("xit", [D, B], F32, kind="ExternalInput")
    out = nc.dram_tensor("out", [B, D], F32, kind="ExternalOutput")
    if debug:
        dbg_dacc = nc.dram_tensor("dbg_dacc", [P, B // P], F32, kind="ExternalOutput")
        dbg_acc0 = nc.dram_tensor("dbg_acc0", [P, D], F32, kind="ExternalOutput")
        dbg_xar = nc.dram_tensor("dbg_xar", [P, D], F32, kind="ExternalOutput")
        dbg_po = nc.dram_tensor("dbg_po", [P, D], F32, kind="ExternalOutput")

    with tile.TileContext(nc) as tc, ExitStack() as ctx:
        pers = ctx.enter_context(tc.tile_pool(name="pers", bufs=1))
        aith = [pers.tile([P, B], F32R, name=f"aith{k}") for k in range(KT)]
        aitl = [pers.tile([P, B], F32R, name=f"aitl{k}") for k in range(KT)]
        acc = [pers.tile([P, D], F32, name=f"acc{t}") for t in range(MT)]
        gm = pers.tile([P, B], F32, name="gm")
        mxbc = pers.tile([P, B], F32, name="mxbc")
        dacc = pers.tile([P, MT], F32, name="dacc")
        ident = pers.tile([P, P], F32, name="ident")
        ones = pers.tile([P, 1], F32R, name="ones")
        dstg = pers.tile([P, P], F32, name="dstg")
        make_identity(nc, ident[:])
        nc.gpsimd.memset(dstg[:], 0.0)
        nc.gpsimd.memset(dstg[:, 0:1], 1.0)
        # DVE-cast producer keeps the f32r verifier happy
        nc.vector.tensor_copy(ones[:], dstg[:, 0:1])
        nc.gpsimd.memset(dstg[:, 0:1], 0.0)
        for t in range(MT):
            nc.gpsimd.memset(acc[t][:], 0.0)
        nc.gpsimd.memset(dacc[:], 0.0)
        nc.gpsimd.memset(gm[:], -1e30)

        # ---- phase 0: M = Wqs Wk^T ; A_i^T = M^T X_i^T ----
        with ExitStack() as p0:
            mpool = p0.enter_context(tc.tile_pool(name="mpool", bufs=1))
            ps0 = p0.enter_context(tc.tile_pool(name="ps0", bufs=4, space="PSUM"))
            m_t = [mpool.tile([P, D], F32, name=f"m{e}") for e in range(KT)]
            with ExitStack() as pA:
                wpool = pA.enter_context(tc.tile_pool(name="wpool", bufs=1))
                wq_t = [wpool.tile([P, D], F32, name=f"wq{g}") for g in range(KT)]
                wk_t = [wpool.tile([P, D], F32, name=f"wk{g}") for g in range(KT)]
                for g in range(KT):
                    nc.sync.dma_start(wq_t[g][:], wqst.ap()[g * P:(g + 1) * P, :])
                    nc.sync.dma_start(wk_t[g][:], wkt.ap()[g * P:(g + 1) * P, :])
                for e in range(KT):
                    for (lo, hi) in _chunks(D):
                        pm = ps0.tile([P, 512], F32, name=f"pm{e}_{lo}", tag="pm")
                        for g in range(KT):
                            nc.tensor.matmul(pm[:, :hi - lo], wq_t[g][:, e * P:(e + 1) * P],
                                             wk_t[g][:, lo:hi], start=(g == 0), stop=(g == KT - 1))
                        nc.scalar.copy(m_t[e][:, lo:hi], pm[:, :hi - lo])
            with ExitStack() as pB:
                xpool = pB.enter_context(tc.tile_pool(name="xpool", bufs=1))
                xi_t = [xpool.tile([P, B], F32, name=f"xi{e}") for e in range(KT)]
                for g in range(KT):
                    nc.sync.dma_start(xi_t[g][:], xit.ap()[g * P:(g + 1) * P, :])
                for d in range(KT):
                    for (lo, hi) in _chunks(B):
                        pa = ps0.tile([P, 512], F32, name=f"pa{d}_{lo}", tag="pm")
                        for e in range(KT):
                            nc.tensor.matmul(pa[:, :hi - lo], m_t[e][:, d * P:(d + 1) * P],
                                             xi_t[e][:, lo:hi], start=(e == 0), stop=(e == KT - 1))
                        a_f = xpool.tile([P, 512], F32, name=f"af{d}_{lo}", tag="af", bufs=2)
                        nc.scalar.copy(a_f[:, :hi - lo], pa[:, :hi - lo])
                        # hi = round_f32r(A); lo = round_f32r(A - hi)
                        nc.vector.tensor_copy(aith[d][:, lo:hi], a_f[:, :hi - lo])
                        al_f = xpool.tile([P, 512], F32, name=f"alf{d}_{lo}", tag="alf", bufs=2)
                        nc.vector.tensor_sub(al_f[:, :hi - lo], a_f[:, :hi - lo],
                                             aith[d][:, lo:hi].bitcast(F32))
                        nc.vector.tensor_copy(aitl[d][:, lo:hi], al_f[:, :hi - lo])

        # ---- flash over key super-blocks ----
        sp = ctx.enter_context(tc.tile_pool(name="sp", bufs=6))
        erp = ctx.enter_context(tc.tile_pool(name="erp", bufs=sbn + 1))
        xthp = ctx.enter_context(tc.tile_pool(name="xthp", bufs=sbn))
        xap = ctx.enter_context(tc.tile_pool(name="xap", bufs=sbn))
        stat = ctx.enter_context(tc.tile_pool(name="stat", bufs=2))
        ps_s = ctx.enter_context(tc.tile_pool(name="ps_s", bufs=2, space="PSUM"))
        ps_o = ctx.enter_context(tc.tile_pool(name="ps_o", bufs=2, space="PSUM"))
        ps_d = ctx.enter_context(tc.tile_pool(name="ps_d", bufs=2, space="PSUM"))

        omx = None
        for s in range(NSB):
            js = list(range(s * sbn, (s + 1) * sbn))
            xsplit, ssb, xar = [], [], []
            # prep: DMA the host-prequantized f32r hi/lo X^T blocks
            for j in js:
                xth = xthp.tile([P, D], F32R, name=f"xth{j}", tag="xth")
                nc.sync.dma_start(xth[:], xtjh.ap()[j])
                xtl = xthp.tile([P, D], F32R, name=f"xtl{j}", tag="xtl")
                nc.sync.dma_start(xtl[:], xtjl.ap()[j])
                xsplit.append((xth, xtl))

            # S^T logits: 3 f32r streams per 512-chunk, key-major
            for idx, j in enumerate(js):
                xth, xtl = xsplit[idx]
                s_t = sp.tile([P, B], F32, name=f"s{j}", tag="s")
                for (lo, hi) in _chunks(B):
                    pss = ps_s.tile([P, 512], F32, name=f"pss{j}_{lo}", tag="pss")
                    nmm = 3 * KT
                    i = 0
                    for k in range(KT):
                        kc = slice(k * P, (k + 1) * P)
                        for la, rb in ((xth, aith[k]), (xth, aitl[k]), (xtl, aith[k])):
                            nc.tensor.matmul(pss[:, :hi - lo], la[:, kc], rb[:, lo:hi],
                                             start=(i == 0), stop=(i == nmm - 1))
                            i += 1
                    nc.scalar.copy(s_t[:, lo:hi], pss[:, :hi - lo])
                    nc.vector.tensor_max(gm[:, lo:hi], gm[:, lo:hi],
                                         s_t[:, lo:hi])
                ssb.append(s_t)

            # per-query-column running max (transpose-reduce gm chunks)
            nmx = stat.tile([P, MT], F32, name=f"nmx{s}", tag="nmx")
            corr = stat.tile([P, MT], F32, name=f"corr{s}", tag="corr")
            for c in range(MT):
                pt = ps_s.tile([P, P], F32, name=f"pt{s}_{c}", tag="pss")
                nc.tensor.transpose(pt[:], gm[:, c * P:(c + 1) * P], ident[:])
                nc.vector.reduce_max(nmx[:, c:c + 1], pt[:], axis=AXX)
            if omx is None:
                nc.vector.memset(corr[:], 0.0)
            else:
                dmx = stat.tile([P, MT], F32, name=f"dmx{s}", tag="dmx")
                nc.vector.tensor_sub(dmx[:], omx[:], nmx[:])
                nc.scalar.activation(corr[:], dmx[:], EXP)
            omx = nmx

            # broadcast nmx (query-major) -> mxbc [P, B] (key-major free)
            ptb = ps_s.tile([P, P], F32, name=f"ptb{s}", tag="pss")
            nc.tensor.transpose(ptb[:MT, :], nmx[:], ident[:])
            mtmp = stat.tile([MT, P], F32, name=f"mtmp{s}", tag="mtmp", bufs=1)
            nc.scalar.copy(mtmp[:], ptb[:MT, :])
            mrow = stat.tile([1, B], F32, name=f"mrow{s}", tag="mrow", bufs=1)
            nc.sync.dma_start(mrow[:].rearrange("a (b c) -> a b c", b=MT),
                              mtmp[:])
            nc.gpsimd.partition_broadcast(mxbc[:], mrow[:])

            # E = exp(S - max): ACT writes f32r directly
            ers = []
            for idx, s_t in enumerate(ssb):
                nc.vector.tensor_sub(s_t[:], s_t[:], mxbc[:])
                er_t = erp.tile([P, B], F32R, name=f"er{s}_{idx}", tag="er")
                nc.scalar.activation(er_t[:], s_t[:], EXP)
                ers.append(er_t)

            # value-side X rows (host f32r)
            for j in js:
                xa_t = xap.tile([P, D], F32R, name=f"xa{j}", tag="xar")
                nc.sync.dma_start(xa_t[:], xa.ap()[j * P:(j + 1) * P, :])
                xar.append(xa_t)

            if debug and s == NSB - 1:
                xtmp = sp.tile([P, D], F32, name="xtmp", tag="s")
                nc.scalar.copy(xtmp[:], xar[1][:].bitcast(F32))
                nc.sync.dma_start(dbg_xar.ap()[:, :], xtmp[:])

            # denominator: ones-vector stationary, er moving -> pd [1, 512]
            pds = []
            for ci, (lo, hi) in enumerate(_chunks(B)):
                pd = ps_d.tile([1, 512], F32, name=f"pd{s}_{ci}", tag="pd")
                for idx in range(sbn):
                    nc.tensor.matmul(pd[:, :hi - lo], ones[:], ers[idx][:][:, lo:hi],
                                     start=(idx == 0), stop=(idx == sbn - 1))
                pds.append(pd)

            # out accumulation: acc = acc*corr + E^T @ X (f32r burst)
            for t in range(MT):
                po = ps_o.tile([P, D], F32, name=f"po{s}_{t}", tag="po")
                for (lo, hi) in _chunks(D):
                    for idx in range(sbn):
                        nc.tensor.matmul(po[:, lo:hi], ers[idx][:][:, t * P:(t + 1) * P],
                                         xar[idx][:][:, lo:hi], start=(idx == 0), stop=(idx == sbn - 1))
                if debug and s == NSB - 1 and t == 0:
                    potmp = sp.tile([P, D], F32, name="potmp", tag="s")
                    nc.scalar.copy(potmp[:], po[:])
                    nc.sync.dma_start(dbg_po.ap()[:, :], potmp[:])
                nc.vector.scalar_tensor_tensor(acc[t][:], acc[t][:],
                                               corr[:, t:t + 1], po[:],
                                               op0=ALU.mult, op1=ALU.add)

            # fold pd [1, B] back to query-major [P, MT] via DMA + transpose
            # (dstg rows MT.. stay zero so the transpose runs full-width)
            drow = stat.tile([1, B], F32, name=f"drow{s}", tag="drow", bufs=1)
            for ci, (lo, hi) in enumerate(_chunks(B)):
                nc.scalar.copy(drow[:, lo:hi], pds[ci][:, :hi - lo])
            nc.sync.dma_start(dstg[:MT, :],
                              drow[:].rearrange("a (b c) -> a b c", b=MT))
            ptd = ps_s.tile([P, P], F32, name=f"ptd{s}", tag="pss")
            nc.tensor.transpose(ptd[:], dstg[:], ident[:])
            nc.vector.tensor_mul(dacc[:], dacc[:], corr[:])
            nc.vector.tensor_add(dacc[:], dacc[:], ptd[:, :MT])

        if debug:
            nc.sync.dma_start(dbg_dacc.ap()[:, :], dacc[:])
            nc.sync.dma_start(dbg_acc0.ap()[:, :], acc[0][:])

        # ---- finalize: divide by the softmax denominator, write out ----
        for t in range(MT):
            rc = stat.tile([P, 1], F32, name=f"rc{t}", tag="rc")
            nc.vector.reciprocal(rc[:], dacc[:, t:t + 1])
            nc.vector.tensor_scalar_mul(acc[t][:], acc[t][:], rc[:])
            nc.sync.dma_start(out.ap()[t * P:(t + 1) * P, :], acc[t][:])

    nc.compile()
    return nc


def prep_inputs(X, Wq, Wk, S, D, n_cores):
    B = S // n_cores
    NT = S // P
    KT = D // P
    X = np.ascontiguousarray(X, np.float32)
    scale = np.float32(1.0 / np.sqrt(D))
    xtj = np.ascontiguousarray(
        X.reshape(NT, P, KT, P).transpose(0, 3, 2, 1).reshape(NT, P, D))
    xtjh = rne12(xtj)
    xtjl = rne12(xtj - xtjh)
    xa = rne12(X)
    wqst = np.ascontiguousarray((np.asarray(Wq, np.float32) * scale).T)
    wkt = np.ascontiguousarray(np.asarray(Wk, np.float32).T)
    xt = X.T
    in_maps = []
    for i in range(n_cores):
        in_maps.append({
            "xtjh": xtjh, "xtjl": xtjl, "xa": xa, "wqst": wqst, "wkt": wkt,
            "xit": np.ascontiguousarray(xt[:, i * B:(i + 1) * B]),
        })
    return in_maps


_CACHE = {}


def _get_kernel(S, D, B, sbn):
    key = (S, D, B, sbn)
    if key not in _CACHE:
        _CACHE[key] = build_core_kernel(S, D, B, sbn=sbn)
    return _CACHE[key]


def kernel(inputs, weight_query, weight_key):
    S, D = inputs.shape
    assert (S, D) == (SEQ, DIM)
    B = S // NCORES
    nc = _get_kernel(S, D, B, SBN)
    in_maps = prep_inputs(inputs, weight_query, weight_key, S, D, NCORES)
    res = run_bass_kernel_spmd(nc, in_maps, core_ids=list(range(NCORES)))
    return np.concatenate([res.results[i]["out"] for i in range(NCORES)], axis=0)


if __name__ == "__main__":
    rng = np.random.default_rng(0)
    X = rng.standard_normal((SEQ, DIM), dtype=np.float32)
    Wq = rng.standard_normal((DIM, DIM), dtype=np.float32)
    Wk = rng.standard_normal((DIM, DIM), dtype=np.float32)
    out = kernel(X, Wq, Wk)
    print(out.shape, out.dtype)
